# revision 1
# baseline (speedup 1.0000x reference)
import os
import sys

import numpy as np


def _ensure_path():
    try:
        import concourse.bass  # noqa: F401
        return
    except ImportError:
        pass
    for p in ("/opt/trn_rl_repo", "/root/.axon_site/_ro/trn_rl_repo"):
        if os.path.isdir(p) and p not in sys.path:
            sys.path.insert(0, p)
    import concourse.bass  # noqa: F401


LAGS = (1, 2, 3, 7, 14, 28)
MAX_LAG = 28
CTX = 168
HP = 24
HID = 512
G = 4 * HID
B = 512
NCORES = 8
BL = B // NCORES  # 64

_F32 = np.float32


def _gate_perm():
    # Gate-output permutation so that the four 512-wide matmul n-tiles are
    # [i0|f0], [i1|f1], [g0|o0], [g1|o1] (x0 = x[:256], x1 = x[256:]).
    # With col-tiling (tile pairs stacked on psum partitions 0:64 / 64:128)
    # the elementwise phase then runs on a folded [128, 256] layout:
    #   p = batch + 64*(hid >= 256), q = hid % 256.
    i = np.arange(0, 512)
    f = 512 + np.arange(0, 512)
    g = 1024 + np.arange(0, 512)
    o = 1536 + np.arange(0, 512)
    return np.concatenate(
        [i[:256], f[:256], i[256:], f[256:], g[:256], o[:256], g[256:], o[256:]]
    )


# ---------------------------------------------------------------------------
# Bass program construction
# ---------------------------------------------------------------------------

_BUILT = {}  # (ctx, hp) -> (nc, runner)

# build-time feature flags (bisect aids; final config ships all-True)
XPROJ = True      # encoder input-projection prefill into next step's psum
BIAS_MM = True    # L1 bias via K=2 matmul prefill (else DVE adds)
HEAD_FOLD = False  # head via DVE reduce + fold matmul (else baseline mms)
BF16_T = True     # bf16 transposes through PSUM (else fp32 like v2)


def _build_nc(ctx, hp):
    _ensure_path()
    import concourse.bacc as bacc
    import concourse.mybir as mybir
    from concourse.tile import TileContext

    dt = mybir.dt.float32
    bf = mybir.dt.bfloat16
    AF = mybir.ActivationFunctionType
    ALU = mybir.AluOpType
    nstep = ctx + hp - 1
    hs = hp - 1
    seq_len = hs + MAX_LAG + 1  # pred slots + initial buffer

    nc = bacc.Bacc()

    # --- dram parameters (per-core shapes) ---
    d_enc = nc.declare_dram_parameter("enc_inT", [11, ctx * BL], bf, isOutput=False)
    d_w0i = nc.declare_dram_parameter("w0i", [11, G], bf, isOutput=False)
    d_w0h = nc.declare_dram_parameter("w0h", [128, 4 * G], bf, isOutput=False)
    d_w1 = nc.declare_dram_parameter("w1", [128, 8 * G], bf, isOutput=False)
    d_ind2 = nc.declare_dram_parameter("ind2", [2, 128], bf, isOutput=False)
    d_b1A = nc.declare_dram_parameter("b1A", [2, 512], bf, isOutput=False)
    d_b1af = nc.declare_dram_parameter("b1af", [128, 512], dt, isOutput=False)
    d_b1bf = nc.declare_dram_parameter("b1bf", [128, 512], dt, isOutput=False)
    d_b1B = nc.declare_dram_parameter("b1B", [2, 512], bf, isOutput=False)
    d_whf = nc.declare_dram_parameter("whf", [128, 256], dt, isOutput=False)
    d_wh4 = nc.declare_dram_parameter("wh4", [128, 4], bf, isOutput=False)
    d_foldm = nc.declare_dram_parameter("foldm", [128, BL], dt, isOutput=False)
    d_bh = nc.declare_dram_parameter("bh64", [BL, 1], dt, isOutput=False)
    d_covs = nc.declare_dram_parameter("covs", [BL, max(3 * hs, 1)], dt, isOutput=False)
    d_buf0 = nc.declare_dram_parameter("buf0", [BL, MAX_LAG + 1], dt, isOutput=False)
    d_scale = nc.declare_dram_parameter("scale", [BL, 1], dt, isOutput=False)
    d_identb = nc.declare_dram_parameter("identb", [128, 128], bf, isOutput=False)
    d_identf = nc.declare_dram_parameter("identf", [128, 128], dt, isOutput=False)
    d_y = nc.declare_dram_parameter("y", [BL, nstep], dt, isOutput=True)

    with TileContext(nc) as tc:
        with (
            tc.sbuf_pool(name="state", bufs=1) as st,
            tc.sbuf_pool(name="work", bufs=2) as wk,
            tc.psum_pool(name="gates", bufs=1) as gp,
            tc.psum_pool(name="tp", bufs=1) as tp,
        ):
            # --- resident tensors ---
            enc = st.tile([11, ctx * BL], bf, name="enc")
            w0i = st.tile([11, G], bf, name="w0i")
            w0h = st.tile([128, 4 * G], bf, name="w0h")
            w1 = st.tile([128, 8 * G], bf, name="w1")
            ind2 = st.tile([2, 128], bf, name="ind2")
            b1A = st.tile([2, 512], bf, name="b1A")
            b1a_f = st.tile([128, 512], dt, name="b1af")
            b1b_f = st.tile([128, 512], dt, name="b1bf")
            b1B = st.tile([2, 512], bf, name="b1B")
            whf = st.tile([128, 256], dt, name="whf")
            wh4 = st.tile([128, 4], bf, name="wh4")
            foldm = st.tile([128, BL], dt, name="foldm")
            bh64 = st.tile([BL, 1], dt, name="bh64")
            covs = st.tile([BL, max(3 * hs, 1)], dt, name="covs")
            scale = st.tile([BL, 1], dt, name="scale")
            identb = st.tile([128, 128], bf, name="identb")
            identf = st.tile([128, 128], dt, name="identf")
            seq = st.tile([BL, seq_len], dt, name="seq")
            xt = st.tile([BL, 11], bf if BF16_T else dt, name="xt")
            xtT = st.tile([11, BL], bf, name="xtT")
            y_all = st.tile([BL, nstep], dt, name="y_all")
            yf = st.tile([128, 1], dt, name="yf")
            h1a = st.tile([128, 128], bf, name="h1a")
            h1b = st.tile([128, 128], bf, name="h1b")
            h2a = st.tile([128, 128], bf, name="h2a")
            h2b = st.tile([128, 128], bf, name="h2b")
            c1 = st.tile([128, 256], dt, name="c1")
            c2 = st.tile([128, 256], dt, name="c2")

            nc.sync.dma_start(enc[:], d_enc[:])
            nc.sync.dma_start(w0i[:], d_w0i[:])
            nc.sync.dma_start(w0h[:], d_w0h[:])
            nc.sync.dma_start(w1[:], d_w1[:])
            nc.sync.dma_start(ind2[:], d_ind2[:])
            nc.sync.dma_start(b1A[:], d_b1A[:])
            nc.sync.dma_start(b1a_f[:], d_b1af[:])
            nc.sync.dma_start(b1b_f[:], d_b1bf[:])
            nc.sync.dma_start(b1B[:], d_b1B[:])
            nc.sync.dma_start(whf[:], d_whf[:])
            nc.sync.dma_start(wh4[:], d_wh4[:])
            nc.sync.dma_start(foldm[:], d_foldm[:])
            nc.sync.dma_start(bh64[:], d_bh[:])
            nc.sync.dma_start(covs[:], d_covs[:])
            nc.sync.dma_start(scale[:], d_scale[:])
            nc.sync.dma_start(identb[:], d_identb[:])
            nc.sync.dma_start(identf[:], d_identf[:])
            nc.sync.dma_start(seq[:, hs : hs + MAX_LAG + 1], d_buf0[:])

            for t_ in (h1a, h1b, h2a, h2b, c1, c2):
                nc.vector.memset(t_[:], 0.0)
            nc.vector.memset(xt[:, 10:11], 1.0)

            def h_chunks(a, b):
                return [a[:, 0:64], b[:, 0:64], a[:, 64:128], b[:, 64:128]]

            w0h_chunks = [w0h[:, k * G : k * G + G] for k in range(4)]
            w1_chunks = [w1[:, k * G : k * G + G] for k in range(8)]

            pending_l0 = {}  # step -> (psA, psB) opened by emit_xproj

            def l0_tiles(t):
                if t in pending_l0:
                    return pending_l0.pop(t)
                p = (t % 2) if XPROJ else 0
                psA = gp.tile([128, 512], dt, tag=f"g0A{p}", name=f"g0A{p}")
                psB = gp.tile([128, 512], dt, tag=f"g0B{p}", name=f"g0B{p}")
                return psA, psB

            def emit_xproj(tt):
                # open the gate-psum accumulation groups for step tt with the
                # (known) input projection: gates += enc_tt^T @ w0i
                psA, psB = l0_tiles(tt)
                xl = enc[:, tt * BL : (tt + 1) * BL]
                nc.tensor.matmul(psA[0:64, :], xl, w0i[:, 0:512], start=True, stop=False, skip_group_check=True)
                nc.tensor.matmul(psA[64:128, :], xl, w0i[:, 512:1024], start=True, stop=False, skip_group_check=True)
                nc.tensor.matmul(psB[0:64, :], xl, w0i[:, 1024:1536], start=True, stop=False, skip_group_check=True)
                nc.tensor.matmul(psB[64:128, :], xl, w0i[:, 1536:2048], start=True, stop=False, skip_group_check=True)
                pending_l0[tt] = (psA, psB)
                return psA, psB

            def emit_l0_h(t, prefilled):
                psA, psB = l0_tiles(t)
                hch = h_chunks(h1a, h1b)
                n = len(hch)
                for j in range(n):
                    s = (not prefilled) and j == 0
                    e = prefilled and j == n - 1
                    nc.tensor.matmul(psA[0:64, :], hch[j], w0h_chunks[j][:, 0:512], start=s, stop=e, skip_group_check=True)
                    nc.tensor.matmul(psA[64:128, :], hch[j], w0h_chunks[j][:, 512:1024], start=s, stop=e, skip_group_check=True)
                for j in range(n):
                    s = (not prefilled) and j == 0
                    e = prefilled and j == n - 1
                    nc.tensor.matmul(psB[0:64, :], hch[j], w0h_chunks[j][:, 1024:1536], start=s, stop=e, skip_group_check=True)
                    nc.tensor.matmul(psB[64:128, :], hch[j], w0h_chunks[j][:, 1536:2048], start=s, stop=e, skip_group_check=True)
                return psA, psB

            def emit_l0_x(psA, psB):
                # decode-time input projection (xtT ready late)
                nc.tensor.matmul(psA[0:64, :], xtT[:], w0i[:, 0:512], start=False, stop=True, skip_group_check=True)
                nc.tensor.matmul(psA[64:128, :], xtT[:], w0i[:, 512:1024], start=False, stop=True, skip_group_check=True)
                nc.tensor.matmul(psB[0:64, :], xtT[:], w0i[:, 1024:1536], start=False, stop=True, skip_group_check=True)
                nc.tensor.matmul(psB[64:128, :], xtT[:], w0i[:, 1536:2048], start=False, stop=True, skip_group_check=True)

            def emit_l1_bias(t):
                if XPROJ:
                    p = t % 2
                    psA = gp.tile([128, 512], dt, tag=f"g0A{p}", name=f"g1A{p}")
                    psB = gp.tile([128, 512], dt, tag=f"g0B{p}", name=f"g1B{p}")
                else:
                    psA = gp.tile([128, 512], dt, tag="g1A", name="g1A")
                    psB = gp.tile([128, 512], dt, tag="g1B", name="g1B")
                if BIAS_MM:
                    nc.tensor.matmul(psA[:], ind2[:], b1A[:], start=True, stop=False, skip_group_check=True)
                    nc.tensor.matmul(psB[:], ind2[:], b1B[:], start=True, stop=False, skip_group_check=True)
                return psA, psB

            def emit_l1_h2part(psA, psB):
                # contraction chunks 4..7 (W_hh1 @ h2(t-1)) — independent of
                # this step's L0 chain, so they fill the PE gap while the
                # L0 elementwise runs.
                lhs = h_chunks(h2a, h2b)
                for k in range(4):
                    j = 4 + k
                    s = (not BIAS_MM) and k == 0
                    nc.tensor.matmul(psA[0:64, :], lhs[k], w1_chunks[j][:, 0:512], start=s, stop=False, skip_group_check=True)
                    nc.tensor.matmul(psA[64:128, :], lhs[k], w1_chunks[j][:, 512:1024], start=s, stop=False, skip_group_check=True)
                for k in range(4):
                    j = 4 + k
                    s = (not BIAS_MM) and k == 0
                    nc.tensor.matmul(psB[0:64, :], lhs[k], w1_chunks[j][:, 1024:1536], start=s, stop=False, skip_group_check=True)
                    nc.tensor.matmul(psB[64:128, :], lhs[k], w1_chunks[j][:, 1536:2048], start=s, stop=False, skip_group_check=True)

            def emit_l1_h1part(psA, psB):
                lhs = h_chunks(h1a, h1b)
                for j in range(4):
                    e = j == 3
                    nc.tensor.matmul(psA[0:64, :], lhs[j], w1_chunks[j][:, 0:512], start=False, stop=e, skip_group_check=True)
                    nc.tensor.matmul(psA[64:128, :], lhs[j], w1_chunks[j][:, 512:1024], start=False, stop=e, skip_group_check=True)
                for j in range(4):
                    e = j == 3
                    nc.tensor.matmul(psB[0:64, :], lhs[j], w1_chunks[j][:, 1024:1536], start=False, stop=e, skip_group_check=True)
                    nc.tensor.matmul(psB[64:128, :], lhs[j], w1_chunks[j][:, 1536:2048], start=False, stop=e, skip_group_check=True)

            def emit_eltwise(psA, psB, c_f, tag):
                # gates layout: psA = [i|f], psB = [g|o] on folded [128, 256+256]
                sif = wk.tile([128, 512], dt, tag=tag + "sif", name=tag + "sif")
                nc.scalar.activation(sif[:], psA[:], AF.Sigmoid)
                tg = wk.tile([128, 256], dt, tag=tag + "tg", name=tag + "tg")
                nc.scalar.activation(tg[:], psB[:, 0:256], AF.Tanh)
                so = wk.tile([128, 256], dt, tag=tag + "so", name=tag + "so")
                nc.scalar.activation(so[:], psB[:, 256:512], AF.Sigmoid)
                t1 = wk.tile([128, 256], dt, tag=tag + "t1", name=tag + "t1")
                nc.vector.tensor_mul(t1[:], sif[:, 256:512], c_f[:])
                t2 = wk.tile([128, 256], dt, tag=tag + "t2", name=tag + "t2")
                nc.vector.tensor_mul(t2[:], sif[:, 0:256], tg[:])
                nc.vector.tensor_add(c_f[:], t1[:], t2[:])
                tch = wk.tile([128, 256], dt, tag=tag + "tc", name=tag + "tc")
                nc.scalar.activation(tch[:], c_f[:], AF.Tanh)
                hf = wk.tile([128, 256], bf if BF16_T else dt, tag=tag + "hf", name=tag + "hf")
                nc.vector.tensor_mul(hf[:], so[:], tch[:])
                return hf

            def emit_hT(hf, hta, htb, tag):
                idt = identb if BF16_T else identf
                tps = tp.tile([128, 256], bf if BF16_T else dt, tag="tps", name=tag + "tps")
                nc.tensor.transpose(tps[:, 0:128], hf[:, 0:128], idt[:])
                nc.tensor.transpose(tps[:, 128:256], hf[:, 128:256], idt[:])
                nc.vector.tensor_copy(hta[:], tps[:, 0:128])
                nc.vector.tensor_copy(htb[:], tps[:, 128:256])

            def emit_head_reduce(hf2, t):
                if not HEAD_FOLD:
                    return
                # yf = sum(hf2 * whf, axis=free); fold across partition halves
                # happens in emit_head_fold via a tiny constant matmul.
                prod = wk.tile([128, 256], dt, tag="hprod", name=f"t{t}hprod")
                nc.vector.tensor_tensor_reduce(
                    prod[:], hf2[:], whf[:], 1.0, 0.0, ALU.mult, ALU.add, yf[:]
                )

            def emit_head_fold(t):
                hd = tp.tile([BL, 1], dt, tag="hd", name=f"t{t}hd")
                if HEAD_FOLD:
                    # y_all[:, t] = foldm^T @ yf + b_head
                    nc.tensor.matmul(hd[:], foldm[:], yf[:, 0:1], start=True, stop=True)
                else:
                    h2c = h_chunks(h2a, h2b)
                    for k in range(4):
                        nc.tensor.matmul(hd[:], h2c[k], wh4[:, k : k + 1], start=(k == 0), stop=(k == 3))
                nc.vector.tensor_scalar_add(y_all[:, t : t + 1], hd[:], bh64[:, 0:1])

            def emit_decode_feed(t):
                # build xt for step t+1 from y_all[:, t] (consumed next step)
                s = t - (ctx - 1)  # decode step that CONSUMES this pred
                col = hs - 1 - s
                nc.vector.tensor_copy(seq[:, col : col + 1], y_all[:, t : t + 1])
                nc.vector.tensor_copy(xt[:, 0:1], y_all[:, t : t + 1])
                nc.vector.tensor_copy(xt[:, 1:4], covs[:, 3 * s : 3 * s + 3])
                for jj, lag in enumerate(LAGS):
                    src = col + lag
                    nc.vector.tensor_copy(xt[:, 4 + jj : 5 + jj], seq[:, src : src + 1])

            if XPROJ:
                # prefill gates(0)
                emit_xproj(0)

            hf1_prev = None
            hf2_prev = None
            for t in range(nstep):
                enc_step = XPROJ and t < ctx
                # P1: L0 h-recurrence matmuls
                psA0, psB0 = emit_l0_h(t, prefilled=enc_step)
                # P2: h2 transpose + head fold of previous step (stalls hidden under P1)
                if hf2_prev is not None:
                    emit_hT(hf2_prev, h2a, h2b, f"t{t-1}h2")
                    emit_head_fold(t - 1)
                    if t - 1 >= ctx - 1:
                        emit_decode_feed(t - 1)
                # P3: next-step input projection (encoder) / this-step x (decode)
                if enc_step:
                    if t + 1 < ctx:
                        emit_xproj(t + 1)
                elif t < ctx:
                    # XPROJ off: inline encoder input projection
                    xl = enc[:, t * BL : (t + 1) * BL]
                    nc.tensor.matmul(psA0[0:64, :], xl, w0i[:, 0:512], start=False, stop=True, skip_group_check=True)
                    nc.tensor.matmul(psA0[64:128, :], xl, w0i[:, 512:1024], start=False, stop=True, skip_group_check=True)
                    nc.tensor.matmul(psB0[0:64, :], xl, w0i[:, 1024:1536], start=False, stop=True, skip_group_check=True)
                    nc.tensor.matmul(psB0[64:128, :], xl, w0i[:, 1536:2048], start=False, stop=True, skip_group_check=True)
                else:
                    xps = tp.tile([11, BL], bf if BF16_T else dt, tag="xps", name=f"t{t}xps")
                    nc.tensor.transpose(xps[:], xt[:], (identb if BF16_T else identf)[0:BL, 0:BL])
                    nc.vector.tensor_copy(xtT[:], xps[:])
                    emit_l0_x(psA0, psB0)
                # L0 elementwise chain (registers the gate-bank readers
                # BEFORE the L1 bias prefill reuses those banks)
                hf1 = emit_eltwise(psA0, psB0, c1, "l0")
                # P4: L1 bias prefill + h2-half of the L1 contraction
                # (fills the PE gap while the L0 elementwise chain runs)
                psA1, psB1 = emit_l1_bias(t)
                emit_l1_h2part(psA1, psB1)
                # P5: h1 transpose (the step's critical join)
                emit_hT(hf1, h1a, h1b, f"t{t}h1")
                # P6: L1 h1-half
                emit_l1_h1part(psA1, psB1)
                if not BIAS_MM:
                    nc.vector.tensor_add(psA1[:], psA1[:], b1a_f[:])
                    nc.vector.tensor_add(psB1[:], psB1[:], b1b_f[:])
                hf2 = emit_eltwise(psA1, psB1, c2, "l1")
                emit_head_reduce(hf2, t)
                hf1_prev, hf2_prev = hf1, hf2

            if not HEAD_FOLD:
                emit_hT(hf2_prev, h2a, h2b, "final_h2")
            emit_head_fold(nstep - 1)

            nc.vector.tensor_scalar_mul(y_all[:], y_all[:], scale[:, 0:1])
            nc.sync.dma_start(d_y[:], y_all[:])

    nc.finalize()
    return nc


# ---------------------------------------------------------------------------
# Persistent PJRT runner (mirrors bass2jax.run_bass_via_pjrt, but cached so
# repeated calls do not re-trace / re-compile)
# ---------------------------------------------------------------------------


def _make_runner(nc):
    _ensure_path()
    import jax
    from jax.experimental.shard_map import shard_map
    from jax.sharding import Mesh, PartitionSpec

    import concourse.mybir as mybir
    from concourse import bass2jax

    bass2jax.install_neuronx_cc_hook()

    partition_name = nc.partition_id_tensor.name if nc.partition_id_tensor else None
    in_names, out_names, out_avals, zero_shapes = [], [], [], []
    for alloc in nc.m.functions[0].allocations:
        if not isinstance(alloc, mybir.MemoryLocationSet):
            continue
        name = alloc.memorylocations[0].name
        if alloc.kind == "ExternalInput":
            if name != partition_name:
                in_names.append(name)
        elif alloc.kind == "ExternalOutput":
            out_names.append(name)
            shape = tuple(alloc.tensor_shape)
            dtype = mybir.dt.np(alloc.dtype)
            out_avals.append(jax.core.ShapedArray(shape, dtype))
            zero_shapes.append((shape, dtype))
    n_params = len(in_names)
    n_outs = len(out_names)
    all_in = list(in_names) + list(out_names)
    if partition_name is not None:
        all_in.append(partition_name)
    all_in = tuple(all_in)

    def _body(*args):
        operands = list(args)
        if partition_name is not None:
            operands.append(bass2jax.partition_id_tensor())
        outs = bass2jax._bass_exec_p.bind(
            *operands,
            out_avals=tuple(out_avals),
            in_names=all_in,
            out_names=tuple(out_names),
            lowering_input_output_aliases=(),
            sim_require_finite=True,
            sim_require_nnan=True,
            nc=nc,
        )
        return tuple(outs)

    devices = jax.devices()[:NCORES]
    assert len(devices) == NCORES, f"need {NCORES} devices, got {len(jax.devices())}"
    mesh = Mesh(np.asarray(devices), ("core",))
    in_specs = (PartitionSpec("core"),) * (n_params + n_outs)
    out_specs = (PartitionSpec("core"),) * n_outs
    donate = tuple(range(n_params, n_params + n_outs))
    sharded = jax.jit(
        shard_map(_body, mesh=mesh, in_specs=in_specs, out_specs=out_specs, check_rep=False),
        donate_argnums=donate,
        keep_unused=True,
    )

    from jax.sharding import NamedSharding

    sharding = NamedSharding(mesh, PartitionSpec("core"))

    def prepare(in_maps):
        """device_put the concatenated inputs once; reuse across timed calls."""
        concat_in = [
            np.concatenate([np.asarray(in_maps[c][nm]) for c in range(NCORES)], axis=0)
            for nm in in_names
        ]
        return [jax.device_put(a, sharding) for a in concat_in]

    def run_prepared(dev_in):
        concat_zeros = [
            jax.device_put(np.zeros((NCORES * s[0],) + s[1:], d), sharding)
            for (s, d) in zero_shapes
        ]
        out_arrs = sharded(*dev_in, *concat_zeros)
        jax.block_until_ready(out_arrs)
        return out_arrs

    def make_zeros():
        return [
            jax.device_put(np.zeros((NCORES * s[0],) + s[1:], d), sharding)
            for (s, d) in zero_shapes
        ]

    def dispatch(dev_in, zeros):
        return sharded(*dev_in, *zeros)

    def run(in_maps):
        out_arrs = run_prepared(prepare(in_maps))
        outs = []
        for c in range(NCORES):
            outs.append(
                {
                    nm: np.asarray(out_arrs[i]).reshape((NCORES,) + zero_shapes[i][0])[c]
                    for i, nm in enumerate(out_names)
                }
            )
        return outs

    run.prepare = prepare
    run.run_prepared = run_prepared
    run.make_zeros = make_zeros
    run.dispatch = dispatch
    return run


def _get_runner(ctx, hp):
    key = (ctx, hp)
    if key not in _BUILT:
        nc = _build_nc(ctx, hp)
        _BUILT[key] = _make_runner(nc)
    return _BUILT[key]


# ---------------------------------------------------------------------------
# Host-side prep + full model entry
# ---------------------------------------------------------------------------


def _prep_in_maps(X, pad_mask, hp, ctx, W_ih0, W_hh0, b0, W_ih1, W_hh1, b1, W_head, b_head):
    import ml_dtypes

    f32 = _F32
    bf16 = np.dtype(ml_dtypes.bfloat16)
    X = np.asarray(X, f32).copy()
    pad_mask = np.asarray(pad_mask)
    B_, L_, _ = X.shape
    hs = hp - 1
    X[:, L_ - hs :, 0] = 0.0
    past = X[:, : L_ - hs, 0][:, ::-1]  # [B, MAX_LAG+ctx] newest-first
    Xs = X[:, MAX_LAG:]  # [B, ctx+hs, 3]
    m = pad_mask[:, MAX_LAG:][:, :ctx].astype(f32)
    scale = (np.abs(Xs[:, :ctx, 0]) * m).sum(1) / np.maximum(m.sum(1), 1.0)
    scale = np.maximum(scale, 1e-3).astype(f32)  # [B]
    pastn = (past / scale[:, None]).astype(f32)
    logs = np.log(scale)
    tgt = Xs[:, :, 0] / scale[:, None]

    idx = (ctx - 1 - np.arange(ctx))[:, None] + np.asarray(LAGS)[None, :]
    lags = pastn[:, idx]  # [B, ctx, 6]
    enc = np.concatenate(
        [
            tgt[:, :ctx, None],
            Xs[:, :ctx, 1:3],
            np.broadcast_to(logs[:, None, None], (B_, ctx, 1)),
            lags,
            np.ones((B_, ctx, 1), f32),
        ],
        axis=2,
    ).astype(f32)  # [B, ctx, 11]
    covs = np.concatenate(
        [Xs[:, ctx:, 1:3], np.broadcast_to(logs[:, None, None], (B_, hs, 1))], axis=2
    ).astype(f32)  # [B, hs, 3]
    buf0 = pastn[:, : MAX_LAG + 1]

    perm = _gate_perm()
    W_ih0 = np.asarray(W_ih0, f32)[perm]
    W_hh0 = np.asarray(W_hh0, f32)[perm]
    b0p = np.asarray(b0, f32)[perm]
    W_ih1 = np.asarray(W_ih1, f32)[perm]
    W_hh1 = np.asarray(W_hh1, f32)[perm]
    b1p = np.asarray(b1, f32)[perm]
    W_head = np.asarray(W_head, f32)
    b_head = np.asarray(b_head, f32)

    w0i = np.ascontiguousarray(np.concatenate([W_ih0.T, b0p[None, :]], 0))  # [11, G]
    W0hT = W_hh0.T  # [512, G]
    w0h = np.ascontiguousarray(np.concatenate([W0hT[128 * k : 128 * (k + 1)] for k in range(4)], 1))
    W1T = np.concatenate([W_ih1.T, W_hh1.T], 0)  # [1024, G]
    w1 = np.ascontiguousarray(np.concatenate([W1T[128 * k : 128 * (k + 1)] for k in range(8)], 1))

    # L1 bias prefill operands: gates += ind2^T @ b1{A,B}
    ind2 = np.zeros((2, 128), f32)
    ind2[0, 0:64] = 1.0
    ind2[1, 64:128] = 1.0
    b1A = np.stack([b1p[0:512], b1p[512:1024]], 0)  # [2, 512]
    b1B = np.stack([b1p[1024:1536], b1p[1536:2048]], 0)
    b1af = np.empty((128, 512), f32)
    b1af[0:64] = b1p[0:512]
    b1af[64:128] = b1p[512:1024]
    b1bf = np.empty((128, 512), f32)
    b1bf[0:64] = b1p[1024:1536]
    b1bf[64:128] = b1p[1536:2048]

    # head weights in the folded hf2 layout: whf[p, q] = W_head[q + 256*(p>=64)]
    whf = np.empty((128, 256), f32)
    whf[0:64] = W_head[0:256, 0][None, :]
    whf[64:128] = W_head[256:512, 0][None, :]

    bh64 = np.full((BL, 1), float(b_head[0]), f32)
    identb = np.eye(128, dtype=f32)
    foldm = np.zeros((128, BL), f32)
    foldm[np.arange(BL), np.arange(BL)] = 1.0
    foldm[BL + np.arange(BL), np.arange(BL)] = 1.0

    in_maps = []
    for c in range(NCORES):
        sl = slice(c * BL, (c + 1) * BL)
        enc_inT = np.ascontiguousarray(enc[sl].transpose(2, 1, 0).reshape(11, ctx * BL))
        in_maps.append(
            {
                "enc_inT": enc_inT.astype(bf16),
                "w0i": w0i.astype(bf16),
                "w0h": w0h.astype(bf16),
                "w1": w1.astype(bf16),
                "ind2": ind2.astype(bf16),
                "b1A": b1A.astype(bf16),
                "b1B": b1B.astype(bf16),
                "b1af": b1af,
                "b1bf": b1bf,
                "whf": whf,
                "wh4": np.stack([W_head[128 * k : 128 * (k + 1), 0] for k in range(4)], 1).astype(bf16),
                "foldm": foldm,
                "bh64": bh64,
                "covs": np.ascontiguousarray(covs[sl].reshape(BL, max(3 * hs, 1))),
                "buf0": np.ascontiguousarray(buf0[sl]),
                "scale": np.ascontiguousarray(scale[sl, None]),
                "identb": identb.astype(bf16),
                "identf": identb,
            }
        )
    return in_maps, scale


def run_model(X, pad_mask, H, context_length, W_ih0, W_hh0, b0, W_ih1, W_hh1, b1, W_head, b_head):
    hp = int(H)
    ctx = int(context_length)
    in_maps, _ = _prep_in_maps(
        X, pad_mask, hp, ctx, W_ih0, W_hh0, b0, W_ih1, W_hh1, b1, W_head, b_head
    )
    run = _get_runner(ctx, hp)
    outs = run(in_maps)
    y = np.concatenate([outs[c]["y"] for c in range(NCORES)], axis=0)  # [B, nstep]
    return y[:, :, None].astype(_F32)


def kernel(**inputs):
    return run_model(
        inputs["X"],
        inputs["pad_mask"],
        inputs["H"],
        inputs["context_length"],
        inputs["W_ih0"],
        inputs["W_hh0"],
        inputs["b0"],
        inputs["W_ih1"],
        inputs["W_hh1"],
        inputs["b1"],
        inputs["W_head"],
        inputs["b_head"],
    )



# revision 36
# speedup vs baseline: 1.0676x; 1.0676x over previous
import os
import sys

import numpy as np


def _ensure_path():
    try:
        import concourse.bass  # noqa: F401
        return
    except ImportError:
        pass
    for p in ("/opt/trn_rl_repo", "/root/.axon_site/_ro/trn_rl_repo"):
        if os.path.isdir(p) and p not in sys.path:
            sys.path.insert(0, p)
    import concourse.bass  # noqa: F401


LAGS = (1, 2, 3, 7, 14, 28)
MAX_LAG = 28
CTX = 168
HP = 24
HID = 512
G = 4 * HID
B = 512
NCORES = 8
BL = B // NCORES  # 64

_F32 = np.float32


def _gate_perm():
    # Gate-output permutation so that the four 512-wide matmul n-tiles are
    # [i0|f0], [i1|f1], [g0|o0], [g1|o1] (x0 = x[:256], x1 = x[256:]).
    # With col-tiling (tile pairs stacked on psum partitions 0:64 / 64:128)
    # the elementwise phase then runs on a folded [128, 256] layout:
    #   p = batch + 64*(hid >= 256), q = hid % 256.
    i = np.arange(0, 512)
    f = 512 + np.arange(0, 512)
    g = 1024 + np.arange(0, 512)
    o = 1536 + np.arange(0, 512)
    return np.concatenate(
        [i[:256], f[:256], i[256:], f[256:], g[:256], o[:256], g[256:], o[256:]]
    )


# ---------------------------------------------------------------------------
# Bass program construction
# ---------------------------------------------------------------------------

_BUILT = {}  # (ctx, hp) -> (nc, runner)

# build-time feature flags (bisect aids; final config ships all-True)
XPROJ = True      # encoder input-projection prefill into next step's psum
BIAS_MM = True    # L1 bias via K=2 matmul prefill (else DVE adds)
HEAD_FOLD = False  # head via DVE reduce + fold matmul (else baseline mms)
BF16_T = True     # bf16 transposes through PSUM (else fp32 like v2)


def _build_nc(ctx, hp):
    _ensure_path()
    import concourse.bacc as bacc
    import concourse.mybir as mybir
    from concourse.tile import TileContext

    dt = mybir.dt.float32
    bf = mybir.dt.bfloat16
    AF = mybir.ActivationFunctionType
    ALU = mybir.AluOpType
    nstep = ctx + hp - 1
    hs = hp - 1
    seq_len = hs + MAX_LAG + 1  # pred slots + initial buffer

    nc = bacc.Bacc()

    # --- dram parameters (per-core shapes) ---
    d_enc = nc.declare_dram_parameter("enc_inT", [11, ctx * BL], bf, isOutput=False)
    d_w0i = nc.declare_dram_parameter("w0i", [11, G], bf, isOutput=False)
    d_w0h = nc.declare_dram_parameter("w0h", [128, 4 * G], bf, isOutput=False)
    d_w1 = nc.declare_dram_parameter("w1", [128, 8 * G], bf, isOutput=False)
    d_ind2 = nc.declare_dram_parameter("ind2", [2, 128], bf, isOutput=False)
    d_b1A = nc.declare_dram_parameter("b1A", [2, 512], bf, isOutput=False)
    d_b1af = nc.declare_dram_parameter("b1af", [128, 512], dt, isOutput=False)
    d_b1bf = nc.declare_dram_parameter("b1bf", [128, 512], dt, isOutput=False)
    d_b1B = nc.declare_dram_parameter("b1B", [2, 512], bf, isOutput=False)
    d_whf = nc.declare_dram_parameter("whf", [128, 256], dt, isOutput=False)
    d_wh4 = nc.declare_dram_parameter("wh4", [128, 4], bf, isOutput=False)
    d_foldm = nc.declare_dram_parameter("foldm", [128, BL], dt, isOutput=False)
    d_bh = nc.declare_dram_parameter("bh64", [BL, 1], dt, isOutput=False)
    d_covs = nc.declare_dram_parameter("covs", [BL, max(3 * hs, 1)], dt, isOutput=False)
    d_buf0 = nc.declare_dram_parameter("buf0", [BL, MAX_LAG + 1], dt, isOutput=False)
    d_scale = nc.declare_dram_parameter("scale", [BL, 1], dt, isOutput=False)
    d_identb = nc.declare_dram_parameter("identb", [128, 128], bf, isOutput=False)
    d_identf = nc.declare_dram_parameter("identf", [128, 128], dt, isOutput=False)
    d_y = nc.declare_dram_parameter("y", [BL, nstep], dt, isOutput=True)

    with TileContext(nc) as tc:
        with (
            tc.sbuf_pool(name="state", bufs=1) as st,
            tc.sbuf_pool(name="work", bufs=2) as wk,
            tc.psum_pool(name="gates", bufs=1) as gp,
            tc.psum_pool(name="tp", bufs=1) as tp,
        ):
            # --- resident tensors ---
            enc = st.tile([11, ctx * BL], bf, name="enc")
            w0i = st.tile([11, G], bf, name="w0i")
            w0h = st.tile([128, 4 * G], bf, name="w0h")
            w1 = st.tile([128, 8 * G], bf, name="w1")
            ind2 = st.tile([2, 128], bf, name="ind2")
            b1A = st.tile([2, 512], bf, name="b1A")
            b1a_f = st.tile([128, 512], dt, name="b1af")
            b1b_f = st.tile([128, 512], dt, name="b1bf")
            b1B = st.tile([2, 512], bf, name="b1B")
            whf = st.tile([128, 256], dt, name="whf")
            wh4 = st.tile([128, 4], bf, name="wh4")
            foldm = st.tile([128, BL], dt, name="foldm")
            bh64 = st.tile([BL, 1], dt, name="bh64")
            covs = st.tile([BL, max(3 * hs, 1)], dt, name="covs")
            scale = st.tile([BL, 1], dt, name="scale")
            identb = st.tile([128, 128], bf, name="identb")
            identf = st.tile([128, 128], dt, name="identf")
            seq = st.tile([BL, seq_len], dt, name="seq")
            xt = st.tile([BL, 11], bf if BF16_T else dt, name="xt")
            xtT = st.tile([11, BL], bf, name="xtT")
            y_all = st.tile([BL, nstep], dt, name="y_all")
            yf = st.tile([128, 1], dt, name="yf")
            h1a = st.tile([128, 128], bf, name="h1a")
            h1b = st.tile([128, 128], bf, name="h1b")
            h2a = st.tile([128, 128], bf, name="h2a")
            h2b = st.tile([128, 128], bf, name="h2b")
            c1 = st.tile([128, 256], dt, name="c1")
            c2 = st.tile([128, 256], dt, name="c2")

            nc.sync.dma_start(enc[:], d_enc[:])
            nc.sync.dma_start(w0i[:], d_w0i[:])
            nc.sync.dma_start(w0h[:], d_w0h[:])
            nc.sync.dma_start(w1[:], d_w1[:])
            nc.sync.dma_start(ind2[:], d_ind2[:])
            nc.sync.dma_start(b1A[:], d_b1A[:])
            nc.sync.dma_start(b1a_f[:], d_b1af[:])
            nc.sync.dma_start(b1b_f[:], d_b1bf[:])
            nc.sync.dma_start(b1B[:], d_b1B[:])
            nc.sync.dma_start(whf[:], d_whf[:])
            nc.sync.dma_start(wh4[:], d_wh4[:])
            nc.sync.dma_start(foldm[:], d_foldm[:])
            nc.sync.dma_start(bh64[:], d_bh[:])
            nc.sync.dma_start(covs[:], d_covs[:])
            nc.sync.dma_start(scale[:], d_scale[:])
            nc.sync.dma_start(identb[:], d_identb[:])
            nc.sync.dma_start(identf[:], d_identf[:])
            nc.sync.dma_start(seq[:, hs : hs + MAX_LAG + 1], d_buf0[:])

            for t_ in (h1a, h1b, h2a, h2b, c1, c2):
                nc.vector.memset(t_[:], 0.0)
            nc.vector.memset(xt[:, 10:11], 1.0)

            def h_chunks(a, b):
                return [a[:, 0:64], b[:, 0:64], a[:, 64:128], b[:, 64:128]]

            w0h_chunks = [w0h[:, k * G : k * G + G] for k in range(4)]
            w1_chunks = [w1[:, k * G : k * G + G] for k in range(8)]

            pending_l0 = {}  # step -> (psA, psB) opened by emit_xproj

            def l0_tiles(t):
                if t in pending_l0:
                    return pending_l0.pop(t)
                p = (t % 2) if XPROJ else 0
                psA = gp.tile([128, 512], dt, tag=f"g0A{p}", name=f"g0A{p}")
                psB = gp.tile([128, 512], dt, tag=f"g0B{p}", name=f"g0B{p}")
                return psA, psB

            def emit_xproj(tt):
                # open the gate-psum accumulation groups for step tt with the
                # (known) input projection: gates += enc_tt^T @ w0i
                psA, psB = l0_tiles(tt)
                xl = enc[:, tt * BL : (tt + 1) * BL]
                nc.tensor.matmul(psA[0:64, :], xl, w0i[:, 0:512], start=True, stop=False, skip_group_check=True)
                nc.tensor.matmul(psA[64:128, :], xl, w0i[:, 512:1024], start=True, stop=False, skip_group_check=True)
                nc.tensor.matmul(psB[0:64, :], xl, w0i[:, 1024:1536], start=True, stop=False, skip_group_check=True)
                nc.tensor.matmul(psB[64:128, :], xl, w0i[:, 1536:2048], start=True, stop=False, skip_group_check=True)
                pending_l0[tt] = (psA, psB)
                return psA, psB

            def emit_l0_h(t, prefilled):
                psA, psB = l0_tiles(t)
                hch = h_chunks(h1a, h1b)
                n = len(hch)
                for j in range(n):
                    s = (not prefilled) and j == 0
                    e = prefilled and j == n - 1
                    nc.tensor.matmul(psA[0:64, :], hch[j], w0h_chunks[j][:, 0:512], start=s, stop=e, skip_group_check=True)
                    nc.tensor.matmul(psA[64:128, :], hch[j], w0h_chunks[j][:, 512:1024], start=s, stop=e, skip_group_check=True)
                for j in range(n):
                    s = (not prefilled) and j == 0
                    e = prefilled and j == n - 1
                    nc.tensor.matmul(psB[0:64, :], hch[j], w0h_chunks[j][:, 1024:1536], start=s, stop=e, skip_group_check=True)
                    nc.tensor.matmul(psB[64:128, :], hch[j], w0h_chunks[j][:, 1536:2048], start=s, stop=e, skip_group_check=True)
                return psA, psB

            def emit_l0_x(psA, psB):
                # decode-time input projection (xtT ready late)
                nc.tensor.matmul(psA[0:64, :], xtT[:], w0i[:, 0:512], start=False, stop=True, skip_group_check=True)
                nc.tensor.matmul(psA[64:128, :], xtT[:], w0i[:, 512:1024], start=False, stop=True, skip_group_check=True)
                nc.tensor.matmul(psB[0:64, :], xtT[:], w0i[:, 1024:1536], start=False, stop=True, skip_group_check=True)
                nc.tensor.matmul(psB[64:128, :], xtT[:], w0i[:, 1536:2048], start=False, stop=True, skip_group_check=True)

            def emit_l1_bias(t):
                if XPROJ:
                    p = t % 2
                    psA = gp.tile([128, 512], dt, tag=f"g0A{p}", name=f"g1A{p}")
                    psB = gp.tile([128, 512], dt, tag=f"g0B{p}", name=f"g1B{p}")
                else:
                    psA = gp.tile([128, 512], dt, tag="g1A", name="g1A")
                    psB = gp.tile([128, 512], dt, tag="g1B", name="g1B")
                if BIAS_MM:
                    nc.tensor.matmul(psA[:], ind2[:], b1A[:], start=True, stop=False, skip_group_check=True)
                    nc.tensor.matmul(psB[:], ind2[:], b1B[:], start=True, stop=False, skip_group_check=True)
                return psA, psB

            def emit_l1_h2part(psA, psB):
                # contraction chunks 4..7 (W_hh1 @ h2(t-1)) — independent of
                # this step's L0 chain, so they fill the PE gap while the
                # L0 elementwise runs.
                lhs = h_chunks(h2a, h2b)
                for k in range(4):
                    j = 4 + k
                    s = (not BIAS_MM) and k == 0
                    nc.tensor.matmul(psA[0:64, :], lhs[k], w1_chunks[j][:, 0:512], start=s, stop=False, skip_group_check=True)
                    nc.tensor.matmul(psA[64:128, :], lhs[k], w1_chunks[j][:, 512:1024], start=s, stop=False, skip_group_check=True)
                for k in range(4):
                    j = 4 + k
                    s = (not BIAS_MM) and k == 0
                    nc.tensor.matmul(psB[0:64, :], lhs[k], w1_chunks[j][:, 1024:1536], start=s, stop=False, skip_group_check=True)
                    nc.tensor.matmul(psB[64:128, :], lhs[k], w1_chunks[j][:, 1536:2048], start=s, stop=False, skip_group_check=True)

            def emit_l1_h1part(psA, psB):
                lhs = h_chunks(h1a, h1b)
                for j in range(4):
                    e = j == 3
                    nc.tensor.matmul(psA[0:64, :], lhs[j], w1_chunks[j][:, 0:512], start=False, stop=e, skip_group_check=True)
                    nc.tensor.matmul(psA[64:128, :], lhs[j], w1_chunks[j][:, 512:1024], start=False, stop=e, skip_group_check=True)
                for j in range(4):
                    e = j == 3
                    nc.tensor.matmul(psB[0:64, :], lhs[j], w1_chunks[j][:, 1024:1536], start=False, stop=e, skip_group_check=True)
                    nc.tensor.matmul(psB[64:128, :], lhs[j], w1_chunks[j][:, 1536:2048], start=False, stop=e, skip_group_check=True)

            def emit_eltwise(psA, psB, c_f, tag):
                # gates layout: psA = [i|f], psB = [g|o] on folded [128, 256+256]
                sif = wk.tile([128, 512], dt, tag=tag + "sif", name=tag + "sif")
                nc.scalar.activation(sif[:], psA[:], AF.Sigmoid)
                tg = wk.tile([128, 256], dt, tag=tag + "tg", name=tag + "tg")
                nc.scalar.activation(tg[:], psB[:, 0:256], AF.Tanh)
                so = wk.tile([128, 256], dt, tag=tag + "so", name=tag + "so")
                nc.scalar.activation(so[:], psB[:, 256:512], AF.Sigmoid)
                t1 = wk.tile([128, 256], dt, tag=tag + "t1", name=tag + "t1")
                nc.vector.tensor_mul(t1[:], sif[:, 256:512], c_f[:])
                t2 = wk.tile([128, 256], dt, tag=tag + "t2", name=tag + "t2")
                nc.vector.tensor_mul(t2[:], sif[:, 0:256], tg[:])
                nc.vector.tensor_add(c_f[:], t1[:], t2[:])
                tch = wk.tile([128, 256], dt, tag=tag + "tc", name=tag + "tc")
                nc.scalar.activation(tch[:], c_f[:], AF.Tanh)
                hf = wk.tile([128, 256], bf if BF16_T else dt, tag=tag + "hf", name=tag + "hf")
                nc.vector.tensor_mul(hf[:], so[:], tch[:])
                return hf

            def emit_hT(hf, hta, htb, tag):
                idt = identb if BF16_T else identf
                tps = tp.tile([128, 256], bf if BF16_T else dt, tag="tps", name=tag + "tps")
                nc.tensor.transpose(tps[:, 0:128], hf[:, 0:128], idt[:])
                nc.tensor.transpose(tps[:, 128:256], hf[:, 128:256], idt[:])
                nc.vector.tensor_copy(hta[:], tps[:, 0:128])
                nc.vector.tensor_copy(htb[:], tps[:, 128:256])

            def emit_head_reduce(hf2, t):
                if not HEAD_FOLD:
                    return
                # yf = sum(hf2 * whf, axis=free); fold across partition halves
                # happens in emit_head_fold via a tiny constant matmul.
                prod = wk.tile([128, 256], dt, tag="hprod", name=f"t{t}hprod")
                nc.vector.tensor_tensor_reduce(
                    prod[:], hf2[:], whf[:], 1.0, 0.0, ALU.mult, ALU.add, yf[:]
                )

            def emit_head_fold(t):
                hd = tp.tile([BL, 1], dt, tag="hd", name=f"t{t}hd")
                if HEAD_FOLD:
                    # y_all[:, t] = foldm^T @ yf + b_head
                    nc.tensor.matmul(hd[:], foldm[:], yf[:, 0:1], start=True, stop=True)
                else:
                    h2c = h_chunks(h2a, h2b)
                    for k in range(4):
                        nc.tensor.matmul(hd[:], h2c[k], wh4[:, k : k + 1], start=(k == 0), stop=(k == 3))
                nc.vector.tensor_scalar_add(y_all[:, t : t + 1], hd[:], bh64[:, 0:1])

            def emit_decode_feed(t):
                # build xt for step t+1 from y_all[:, t] (consumed next step)
                s = t - (ctx - 1)  # decode step that CONSUMES this pred
                col = hs - 1 - s
                nc.vector.tensor_copy(seq[:, col : col + 1], y_all[:, t : t + 1])
                nc.vector.tensor_copy(xt[:, 0:1], y_all[:, t : t + 1])
                nc.vector.tensor_copy(xt[:, 1:4], covs[:, 3 * s : 3 * s + 3])
                for jj, lag in enumerate(LAGS):
                    src = col + lag
                    nc.vector.tensor_copy(xt[:, 4 + jj : 5 + jj], seq[:, src : src + 1])

            if XPROJ:
                # prefill gates(0)
                emit_xproj(0)

            hf1_prev = None
            hf2_prev = None
            for t in range(nstep):
                enc_step = XPROJ and t < ctx
                # P1: L0 h-recurrence matmuls
                psA0, psB0 = emit_l0_h(t, prefilled=enc_step)
                # P2: h2 transpose + head fold of previous step (stalls hidden under P1)
                if hf2_prev is not None:
                    emit_hT(hf2_prev, h2a, h2b, f"t{t-1}h2")
                    emit_head_fold(t - 1)
                    if t - 1 >= ctx - 1:
                        emit_decode_feed(t - 1)
                # P3: next-step input projection (encoder) / this-step x (decode)
                if enc_step:
                    if t + 1 < ctx:
                        emit_xproj(t + 1)
                elif t < ctx:
                    # XPROJ off: inline encoder input projection
                    xl = enc[:, t * BL : (t + 1) * BL]
                    nc.tensor.matmul(psA0[0:64, :], xl, w0i[:, 0:512], start=False, stop=True, skip_group_check=True)
                    nc.tensor.matmul(psA0[64:128, :], xl, w0i[:, 512:1024], start=False, stop=True, skip_group_check=True)
                    nc.tensor.matmul(psB0[0:64, :], xl, w0i[:, 1024:1536], start=False, stop=True, skip_group_check=True)
                    nc.tensor.matmul(psB0[64:128, :], xl, w0i[:, 1536:2048], start=False, stop=True, skip_group_check=True)
                else:
                    xps = tp.tile([11, BL], bf if BF16_T else dt, tag="xps", name=f"t{t}xps")
                    nc.tensor.transpose(xps[:], xt[:], (identb if BF16_T else identf)[0:BL, 0:BL])
                    nc.vector.tensor_copy(xtT[:], xps[:])
                    emit_l0_x(psA0, psB0)
                # L0 elementwise chain (registers the gate-bank readers
                # BEFORE the L1 bias prefill reuses those banks)
                hf1 = emit_eltwise(psA0, psB0, c1, "l0")
                # P4: L1 bias prefill + h2-half of the L1 contraction
                # (fills the PE gap while the L0 elementwise chain runs)
                psA1, psB1 = emit_l1_bias(t)
                emit_l1_h2part(psA1, psB1)
                # P5: h1 transpose (the step's critical join)
                emit_hT(hf1, h1a, h1b, f"t{t}h1")
                # P6: L1 h1-half
                emit_l1_h1part(psA1, psB1)
                if not BIAS_MM:
                    nc.vector.tensor_add(psA1[:], psA1[:], b1a_f[:])
                    nc.vector.tensor_add(psB1[:], psB1[:], b1b_f[:])
                hf2 = emit_eltwise(psA1, psB1, c2, "l1")
                emit_head_reduce(hf2, t)
                hf1_prev, hf2_prev = hf1, hf2

            if not HEAD_FOLD:
                emit_hT(hf2_prev, h2a, h2b, "final_h2")
            emit_head_fold(nstep - 1)

            nc.vector.tensor_scalar_mul(y_all[:], y_all[:], scale[:, 0:1])
            nc.sync.dma_start(d_y[:], y_all[:])

    nc.finalize()
    return nc


# ---------------------------------------------------------------------------
# Persistent PJRT runner (mirrors bass2jax.run_bass_via_pjrt, but cached so
# repeated calls do not re-trace / re-compile)
# ---------------------------------------------------------------------------


def _make_runner(nc):
    _ensure_path()
    import jax
    from jax.experimental.shard_map import shard_map
    from jax.sharding import Mesh, PartitionSpec

    import concourse.mybir as mybir
    from concourse import bass2jax

    bass2jax.install_neuronx_cc_hook()

    partition_name = nc.partition_id_tensor.name if nc.partition_id_tensor else None
    in_names, out_names, out_avals, zero_shapes = [], [], [], []
    for alloc in nc.m.functions[0].allocations:
        if not isinstance(alloc, mybir.MemoryLocationSet):
            continue
        name = alloc.memorylocations[0].name
        if alloc.kind == "ExternalInput":
            if name != partition_name:
                in_names.append(name)
        elif alloc.kind == "ExternalOutput":
            out_names.append(name)
            shape = tuple(alloc.tensor_shape)
            dtype = mybir.dt.np(alloc.dtype)
            out_avals.append(jax.core.ShapedArray(shape, dtype))
            zero_shapes.append((shape, dtype))
    n_params = len(in_names)
    n_outs = len(out_names)
    all_in = list(in_names) + list(out_names)
    if partition_name is not None:
        all_in.append(partition_name)
    all_in = tuple(all_in)

    def _body(*args):
        operands = list(args)
        if partition_name is not None:
            operands.append(bass2jax.partition_id_tensor())
        outs = bass2jax._bass_exec_p.bind(
            *operands,
            out_avals=tuple(out_avals),
            in_names=all_in,
            out_names=tuple(out_names),
            lowering_input_output_aliases=(),
            sim_require_finite=True,
            sim_require_nnan=True,
            nc=nc,
        )
        return tuple(outs)

    devices = jax.devices()[:NCORES]
    assert len(devices) == NCORES, f"need {NCORES} devices, got {len(jax.devices())}"
    mesh = Mesh(np.asarray(devices), ("core",))
    in_specs = (PartitionSpec("core"),) * (n_params + n_outs)
    out_specs = (PartitionSpec("core"),) * n_outs
    donate = tuple(range(n_params, n_params + n_outs))
    sharded = jax.jit(
        shard_map(_body, mesh=mesh, in_specs=in_specs, out_specs=out_specs, check_rep=False),
        donate_argnums=donate,
        keep_unused=True,
    )

    from jax.sharding import NamedSharding

    sharding = NamedSharding(mesh, PartitionSpec("core"))

    def prepare(in_maps):
        """device_put the concatenated inputs once; reuse across timed calls."""
        concat_in = [
            np.concatenate([np.asarray(in_maps[c][nm]) for c in range(NCORES)], axis=0)
            for nm in in_names
        ]
        return [jax.device_put(a, sharding) for a in concat_in]

    def run_prepared(dev_in):
        concat_zeros = [
            jax.device_put(np.zeros((NCORES * s[0],) + s[1:], d), sharding)
            for (s, d) in zero_shapes
        ]
        out_arrs = sharded(*dev_in, *concat_zeros)
        jax.block_until_ready(out_arrs)
        return out_arrs

    def make_zeros():
        return [
            jax.device_put(np.zeros((NCORES * s[0],) + s[1:], d), sharding)
            for (s, d) in zero_shapes
        ]

    def dispatch(dev_in, zeros):
        return sharded(*dev_in, *zeros)

    def run(in_maps):
        out_arrs = run_prepared(prepare(in_maps))
        outs = []
        for c in range(NCORES):
            outs.append(
                {
                    nm: np.asarray(out_arrs[i]).reshape((NCORES,) + zero_shapes[i][0])[c]
                    for i, nm in enumerate(out_names)
                }
            )
        return outs

    run.prepare = prepare
    run.run_prepared = run_prepared
    run.make_zeros = make_zeros
    run.dispatch = dispatch
    return run


def _get_runner(ctx, hp):
    key = (ctx, hp)
    if key not in _BUILT:
        nc = _build_nc(ctx, hp)
        _BUILT[key] = _make_runner(nc)
    return _BUILT[key]


# ---------------------------------------------------------------------------
# Host-side prep + full model entry
# ---------------------------------------------------------------------------


def _prep_in_maps(X, pad_mask, hp, ctx, W_ih0, W_hh0, b0, W_ih1, W_hh1, b1, W_head, b_head):
    import ml_dtypes

    f32 = _F32
    bf16 = np.dtype(ml_dtypes.bfloat16)
    X = np.asarray(X, f32).copy()
    pad_mask = np.asarray(pad_mask)
    B_, L_, _ = X.shape
    hs = hp - 1
    X[:, L_ - hs :, 0] = 0.0
    past = X[:, : L_ - hs, 0][:, ::-1]  # [B, MAX_LAG+ctx] newest-first
    Xs = X[:, MAX_LAG:]  # [B, ctx+hs, 3]
    m = pad_mask[:, MAX_LAG:][:, :ctx].astype(f32)
    scale = (np.abs(Xs[:, :ctx, 0]) * m).sum(1) / np.maximum(m.sum(1), 1.0)
    scale = np.maximum(scale, 1e-3).astype(f32)  # [B]
    pastn = (past / scale[:, None]).astype(f32)
    logs = np.log(scale)
    tgt = Xs[:, :, 0] / scale[:, None]

    idx = (ctx - 1 - np.arange(ctx))[:, None] + np.asarray(LAGS)[None, :]
    lags = pastn[:, idx]  # [B, ctx, 6]
    enc = np.concatenate(
        [
            tgt[:, :ctx, None],
            Xs[:, :ctx, 1:3],
            np.broadcast_to(logs[:, None, None], (B_, ctx, 1)),
            lags,
            np.ones((B_, ctx, 1), f32),
        ],
        axis=2,
    ).astype(f32)  # [B, ctx, 11]
    covs = np.concatenate(
        [Xs[:, ctx:, 1:3], np.broadcast_to(logs[:, None, None], (B_, hs, 1))], axis=2
    ).astype(f32)  # [B, hs, 3]
    buf0 = pastn[:, : MAX_LAG + 1]

    perm = _gate_perm()
    W_ih0 = np.asarray(W_ih0, f32)[perm]
    W_hh0 = np.asarray(W_hh0, f32)[perm]
    b0p = np.asarray(b0, f32)[perm]
    W_ih1 = np.asarray(W_ih1, f32)[perm]
    W_hh1 = np.asarray(W_hh1, f32)[perm]
    b1p = np.asarray(b1, f32)[perm]
    W_head = np.asarray(W_head, f32)
    b_head = np.asarray(b_head, f32)

    w0i = np.ascontiguousarray(np.concatenate([W_ih0.T, b0p[None, :]], 0))  # [11, G]
    W0hT = W_hh0.T  # [512, G]
    w0h = np.ascontiguousarray(np.concatenate([W0hT[128 * k : 128 * (k + 1)] for k in range(4)], 1))
    W1T = np.concatenate([W_ih1.T, W_hh1.T], 0)  # [1024, G]
    w1 = np.ascontiguousarray(np.concatenate([W1T[128 * k : 128 * (k + 1)] for k in range(8)], 1))

    # L1 bias prefill operands: gates += ind2^T @ b1{A,B}
    ind2 = np.zeros((2, 128), f32)
    ind2[0, 0:64] = 1.0
    ind2[1, 64:128] = 1.0
    b1A = np.stack([b1p[0:512], b1p[512:1024]], 0)  # [2, 512]
    b1B = np.stack([b1p[1024:1536], b1p[1536:2048]], 0)
    b1af = np.empty((128, 512), f32)
    b1af[0:64] = b1p[0:512]
    b1af[64:128] = b1p[512:1024]
    b1bf = np.empty((128, 512), f32)
    b1bf[0:64] = b1p[1024:1536]
    b1bf[64:128] = b1p[1536:2048]

    # head weights in the folded hf2 layout: whf[p, q] = W_head[q + 256*(p>=64)]
    whf = np.empty((128, 256), f32)
    whf[0:64] = W_head[0:256, 0][None, :]
    whf[64:128] = W_head[256:512, 0][None, :]

    bh64 = np.full((BL, 1), float(b_head[0]), f32)
    identb = np.eye(128, dtype=f32)
    foldm = np.zeros((128, BL), f32)
    foldm[np.arange(BL), np.arange(BL)] = 1.0
    foldm[BL + np.arange(BL), np.arange(BL)] = 1.0

    in_maps = []
    for c in range(NCORES):
        sl = slice(c * BL, (c + 1) * BL)
        enc_inT = np.ascontiguousarray(enc[sl].transpose(2, 1, 0).reshape(11, ctx * BL))
        in_maps.append(
            {
                "enc_inT": enc_inT.astype(bf16),
                "w0i": w0i.astype(bf16),
                "w0h": w0h.astype(bf16),
                "w1": w1.astype(bf16),
                "ind2": ind2.astype(bf16),
                "b1A": b1A.astype(bf16),
                "b1B": b1B.astype(bf16),
                "b1af": b1af,
                "b1bf": b1bf,
                "whf": whf,
                "wh4": np.stack([W_head[128 * k : 128 * (k + 1), 0] for k in range(4)], 1).astype(bf16),
                "foldm": foldm,
                "bh64": bh64,
                "covs": np.ascontiguousarray(covs[sl].reshape(BL, max(3 * hs, 1))),
                "buf0": np.ascontiguousarray(buf0[sl]),
                "scale": np.ascontiguousarray(scale[sl, None]),
                "identb": identb.astype(bf16),
                "identf": identb,
            }
        )
    return in_maps, scale


def run_model(X, pad_mask, H, context_length, W_ih0, W_hh0, b0, W_ih1, W_hh1, b1, W_head, b_head):
    hp = int(H)
    ctx = int(context_length)
    in_maps, _ = _prep_in_maps(
        X, pad_mask, hp, ctx, W_ih0, W_hh0, b0, W_ih1, W_hh1, b1, W_head, b_head
    )
    run = _get_runner(ctx, hp)
    outs = run(in_maps)
    y = np.concatenate([outs[c]["y"] for c in range(NCORES)], axis=0)  # [B, nstep]
    return y[:, :, None].astype(_F32)


def kernel(**inputs):
    return run_model(
        inputs["X"],
        inputs["pad_mask"],
        inputs["H"],
        inputs["context_length"],
        inputs["W_ih0"],
        inputs["W_hh0"],
        inputs["b0"],
        inputs["W_ih1"],
        inputs["W_hh1"],
        inputs["b1"],
        inputs["W_head"],
        inputs["b_head"],
    )


# revision 38
# speedup vs baseline: 1.3580x; 1.2721x over previous
import os
import sys

import numpy as np


def _ensure_path():
    try:
        import concourse.bass  # noqa: F401
        return
    except ImportError:
        pass
    for p in ("/opt/trn_rl_repo", "/root/.axon_site/_ro/trn_rl_repo"):
        if os.path.isdir(p) and p not in sys.path:
            sys.path.insert(0, p)
    import concourse.bass  # noqa: F401


LAGS = (1, 2, 3, 7, 14, 28)
MAX_LAG = 28
CTX = 168
HP = 24
HID = 512
G = 4 * HID
B = 512
NCORES = 8
BL = B // NCORES  # 64

_F32 = np.float32


def _gate_perm():
    # Gate-output permutation so that the four 512-wide matmul n-tiles are
    # [i0|f0], [i1|f1], [g0|o0], [g1|o1] (x0 = x[:256], x1 = x[256:]).
    # With col-tiling (tile pairs stacked on psum partitions 0:64 / 64:128)
    # the elementwise phase then runs on a folded [128, 256] layout:
    #   p = batch + 64*(hid >= 256), q = hid % 256.
    i = np.arange(0, 512)
    f = 512 + np.arange(0, 512)
    g = 1024 + np.arange(0, 512)
    o = 1536 + np.arange(0, 512)
    return np.concatenate(
        [i[:256], f[:256], i[256:], f[256:], g[:256], o[:256], g[256:], o[256:]]
    )


# ---------------------------------------------------------------------------
# Bass program construction
# ---------------------------------------------------------------------------

_BUILT = {}  # (ctx, hp) -> (nc, runner)

# build-time feature flags (bisect aids; final config ships all-True)
XPROJ = True      # encoder input-projection prefill into next step's psum
BIAS_MM = True    # L1 bias via K=2 matmul prefill (else DVE adds)
HEAD_FOLD = False  # head via DVE reduce + fold matmul (else baseline mms)
BF16_T = True     # bf16 transposes through PSUM (else fp32 like v2)


def _build_nc(ctx, hp):
    _ensure_path()
    import concourse.bacc as bacc
    import concourse.mybir as mybir
    from concourse.tile import TileContext

    dt = mybir.dt.float32
    bf = mybir.dt.bfloat16
    AF = mybir.ActivationFunctionType
    ALU = mybir.AluOpType
    nstep = ctx + hp - 1
    hs = hp - 1
    seq_len = hs + MAX_LAG + 1  # pred slots + initial buffer

    nc = bacc.Bacc()

    # --- dram parameters (per-core shapes) ---
    d_enc = nc.declare_dram_parameter("enc_inT", [11, ctx * BL], bf, isOutput=False)
    d_w0i = nc.declare_dram_parameter("w0i", [11, G], bf, isOutput=False)
    d_w0h = nc.declare_dram_parameter("w0h", [128, 4 * G], bf, isOutput=False)
    d_w1 = nc.declare_dram_parameter("w1", [128, 8 * G], bf, isOutput=False)
    d_ind2 = nc.declare_dram_parameter("ind2", [2, 128], bf, isOutput=False)
    d_b1A = nc.declare_dram_parameter("b1A", [2, 512], bf, isOutput=False)
    d_b1af = nc.declare_dram_parameter("b1af", [128, 512], dt, isOutput=False)
    d_b1bf = nc.declare_dram_parameter("b1bf", [128, 512], dt, isOutput=False)
    d_b1B = nc.declare_dram_parameter("b1B", [2, 512], bf, isOutput=False)
    d_whf = nc.declare_dram_parameter("whf", [128, 256], dt, isOutput=False)
    d_wh4 = nc.declare_dram_parameter("wh4", [128, 4], bf, isOutput=False)
    d_foldm = nc.declare_dram_parameter("foldm", [128, BL], dt, isOutput=False)
    d_bh = nc.declare_dram_parameter("bh64", [BL, 1], dt, isOutput=False)
    d_covs = nc.declare_dram_parameter("covs", [BL, max(3 * hs, 1)], dt, isOutput=False)
    d_buf0 = nc.declare_dram_parameter("buf0", [BL, MAX_LAG + 1], dt, isOutput=False)
    d_scale = nc.declare_dram_parameter("scale", [BL, 1], dt, isOutput=False)
    d_identb = nc.declare_dram_parameter("identb", [128, 128], bf, isOutput=False)
    d_identf = nc.declare_dram_parameter("identf", [128, 128], dt, isOutput=False)
    d_y = nc.declare_dram_parameter("y", [BL, nstep], dt, isOutput=True)

    with TileContext(nc) as tc:
        with (
            tc.sbuf_pool(name="state", bufs=1) as st,
            tc.sbuf_pool(name="work", bufs=2) as wk,
            tc.psum_pool(name="gates", bufs=1) as gp,
            tc.psum_pool(name="tp", bufs=1) as tp,
        ):
            # --- resident tensors ---
            enc = st.tile([11, ctx * BL], bf, name="enc")
            w0i = st.tile([11, G], bf, name="w0i")
            w0h = st.tile([128, 4 * G], bf, name="w0h")
            w1 = st.tile([128, 8 * G], bf, name="w1")
            ind2 = st.tile([2, 128], bf, name="ind2")
            b1A = st.tile([2, 512], bf, name="b1A")
            b1a_f = st.tile([128, 512], dt, name="b1af")
            b1b_f = st.tile([128, 512], dt, name="b1bf")
            b1B = st.tile([2, 512], bf, name="b1B")
            whf = st.tile([128, 256], dt, name="whf")
            wh4 = st.tile([128, 4], bf, name="wh4")
            foldm = st.tile([128, BL], dt, name="foldm")
            bh64 = st.tile([BL, 1], dt, name="bh64")
            covs = st.tile([BL, max(3 * hs, 1)], dt, name="covs")
            scale = st.tile([BL, 1], dt, name="scale")
            identb = st.tile([128, 128], bf, name="identb")
            identf = st.tile([128, 128], dt, name="identf")
            seq = st.tile([BL, seq_len], dt, name="seq")
            xt = st.tile([BL, 11], bf if BF16_T else dt, name="xt")
            xtT = st.tile([11, BL], bf, name="xtT")
            y_all = st.tile([BL, nstep], dt, name="y_all")
            yf = st.tile([128, 1], dt, name="yf")
            h1a = st.tile([128, 128], bf, name="h1a")
            h1b = st.tile([128, 128], bf, name="h1b")
            h2a = st.tile([128, 128], bf, name="h2a")
            h2b = st.tile([128, 128], bf, name="h2b")
            c1 = st.tile([128, 256], dt, name="c1")
            c2 = st.tile([128, 256], dt, name="c2")

            nc.sync.dma_start(enc[:], d_enc[:])
            nc.sync.dma_start(w0i[:], d_w0i[:])
            nc.sync.dma_start(w0h[:], d_w0h[:])
            nc.sync.dma_start(w1[:], d_w1[:])
            nc.sync.dma_start(ind2[:], d_ind2[:])
            nc.sync.dma_start(b1A[:], d_b1A[:])
            nc.sync.dma_start(b1a_f[:], d_b1af[:])
            nc.sync.dma_start(b1b_f[:], d_b1bf[:])
            nc.sync.dma_start(b1B[:], d_b1B[:])
            nc.sync.dma_start(whf[:], d_whf[:])
            nc.sync.dma_start(wh4[:], d_wh4[:])
            nc.sync.dma_start(foldm[:], d_foldm[:])
            nc.sync.dma_start(bh64[:], d_bh[:])
            nc.sync.dma_start(covs[:], d_covs[:])
            nc.sync.dma_start(scale[:], d_scale[:])
            nc.sync.dma_start(identb[:], d_identb[:])
            nc.sync.dma_start(identf[:], d_identf[:])
            nc.sync.dma_start(seq[:, hs : hs + MAX_LAG + 1], d_buf0[:])

            for t_ in (h1a, h1b, h2a, h2b, c1, c2):
                nc.vector.memset(t_[:], 0.0)
            nc.vector.memset(xt[:, 10:11], 1.0)

            def h_chunks(a, b):
                return [a[:, 0:64], b[:, 0:64], a[:, 64:128], b[:, 64:128]]

            w0h_chunks = [w0h[:, k * G : k * G + G] for k in range(4)]
            w1_chunks = [w1[:, k * G : k * G + G] for k in range(8)]

            pending_l0 = {}  # step -> (psA, psB) opened by emit_xproj

            def l0_tiles(t):
                if t in pending_l0:
                    return pending_l0.pop(t)
                p = (t % 2) if XPROJ else 0
                psA = gp.tile([128, 512], dt, tag=f"g0A{p}", name=f"g0A{p}")
                psB = gp.tile([128, 512], dt, tag=f"g0B{p}", name=f"g0B{p}")
                return psA, psB

            def emit_xproj(tt):
                # open the gate-psum accumulation groups for step tt with the
                # (known) input projection: gates += enc_tt^T @ w0i
                psA, psB = l0_tiles(tt)
                xl = enc[:, tt * BL : (tt + 1) * BL]
                nc.tensor.matmul(psA[0:64, :], xl, w0i[:, 0:512], start=True, stop=False, skip_group_check=True)
                nc.tensor.matmul(psA[64:128, :], xl, w0i[:, 512:1024], start=True, stop=False, skip_group_check=True)
                nc.tensor.matmul(psB[0:64, :], xl, w0i[:, 1024:1536], start=True, stop=False, skip_group_check=True)
                nc.tensor.matmul(psB[64:128, :], xl, w0i[:, 1536:2048], start=True, stop=False, skip_group_check=True)
                pending_l0[tt] = (psA, psB)
                return psA, psB

            def emit_l0_h(t, prefilled):
                psA, psB = l0_tiles(t)
                hch = h_chunks(h1a, h1b)
                n = len(hch)
                for j in range(n):
                    s = (not prefilled) and j == 0
                    e = prefilled and j == n - 1
                    nc.tensor.matmul(psA[0:64, :], hch[j], w0h_chunks[j][:, 0:512], start=s, stop=e, skip_group_check=True)
                    nc.tensor.matmul(psA[64:128, :], hch[j], w0h_chunks[j][:, 512:1024], start=s, stop=e, skip_group_check=True)
                for j in range(n):
                    s = (not prefilled) and j == 0
                    e = prefilled and j == n - 1
                    nc.tensor.matmul(psB[0:64, :], hch[j], w0h_chunks[j][:, 1024:1536], start=s, stop=e, skip_group_check=True)
                    nc.tensor.matmul(psB[64:128, :], hch[j], w0h_chunks[j][:, 1536:2048], start=s, stop=e, skip_group_check=True)
                return psA, psB

            def emit_l0_x(psA, psB):
                # decode-time input projection (xtT ready late)
                nc.tensor.matmul(psA[0:64, :], xtT[:], w0i[:, 0:512], start=False, stop=True, skip_group_check=True)
                nc.tensor.matmul(psA[64:128, :], xtT[:], w0i[:, 512:1024], start=False, stop=True, skip_group_check=True)
                nc.tensor.matmul(psB[0:64, :], xtT[:], w0i[:, 1024:1536], start=False, stop=True, skip_group_check=True)
                nc.tensor.matmul(psB[64:128, :], xtT[:], w0i[:, 1536:2048], start=False, stop=True, skip_group_check=True)

            def emit_l1_bias(t):
                # L1 gets its own psum banks so its bias/h2 matmuls need not
                # wait for the L0 eltwise readers of the shared bank.
                psA = gp.tile([128, 512], dt, tag="g1A", name="g1A")
                psB = gp.tile([128, 512], dt, tag="g1B", name="g1B")
                if BIAS_MM:
                    nc.tensor.matmul(psA[:], ind2[:], b1A[:], start=True, stop=False, skip_group_check=True)
                    nc.tensor.matmul(psB[:], ind2[:], b1B[:], start=True, stop=False, skip_group_check=True)
                return psA, psB

            def emit_l1_h2part(psA, psB):
                # contraction chunks 4..7 (W_hh1 @ h2(t-1)) — independent of
                # this step's L0 chain, so they fill the PE gap while the
                # L0 elementwise runs.
                lhs = h_chunks(h2a, h2b)
                for k in range(4):
                    j = 4 + k
                    s = (not BIAS_MM) and k == 0
                    nc.tensor.matmul(psA[0:64, :], lhs[k], w1_chunks[j][:, 0:512], start=s, stop=False, skip_group_check=True)
                    nc.tensor.matmul(psA[64:128, :], lhs[k], w1_chunks[j][:, 512:1024], start=s, stop=False, skip_group_check=True)
                for k in range(4):
                    j = 4 + k
                    s = (not BIAS_MM) and k == 0
                    nc.tensor.matmul(psB[0:64, :], lhs[k], w1_chunks[j][:, 1024:1536], start=s, stop=False, skip_group_check=True)
                    nc.tensor.matmul(psB[64:128, :], lhs[k], w1_chunks[j][:, 1536:2048], start=s, stop=False, skip_group_check=True)

            def emit_l1_h1part(psA, psB):
                lhs = h_chunks(h1a, h1b)
                for j in range(4):
                    e = j == 3
                    nc.tensor.matmul(psA[0:64, :], lhs[j], w1_chunks[j][:, 0:512], start=False, stop=e, skip_group_check=True)
                    nc.tensor.matmul(psA[64:128, :], lhs[j], w1_chunks[j][:, 512:1024], start=False, stop=e, skip_group_check=True)
                for j in range(4):
                    e = j == 3
                    nc.tensor.matmul(psB[0:64, :], lhs[j], w1_chunks[j][:, 1024:1536], start=False, stop=e, skip_group_check=True)
                    nc.tensor.matmul(psB[64:128, :], lhs[j], w1_chunks[j][:, 1536:2048], start=False, stop=e, skip_group_check=True)

            def emit_eltwise(psA, psB, c_f, tag):
                # gates layout: psA = [i|f], psB = [g|o] on folded [128, 256+256]
                sif = wk.tile([128, 512], dt, tag=tag + "sif", name=tag + "sif")
                nc.scalar.activation(sif[:], psA[:], AF.Sigmoid)
                tg = wk.tile([128, 256], dt, tag=tag + "tg", name=tag + "tg")
                nc.scalar.activation(tg[:], psB[:, 0:256], AF.Tanh)
                so = wk.tile([128, 256], dt, tag=tag + "so", name=tag + "so")
                nc.scalar.activation(so[:], psB[:, 256:512], AF.Sigmoid)
                t1 = wk.tile([128, 256], dt, tag=tag + "t1", name=tag + "t1")
                nc.vector.tensor_mul(t1[:], sif[:, 256:512], c_f[:])
                t2 = wk.tile([128, 256], dt, tag=tag + "t2", name=tag + "t2")
                nc.vector.tensor_mul(t2[:], sif[:, 0:256], tg[:])
                nc.vector.tensor_add(c_f[:], t1[:], t2[:])
                tch = wk.tile([128, 256], dt, tag=tag + "tc", name=tag + "tc")
                nc.scalar.activation(tch[:], c_f[:], AF.Tanh)
                hf = wk.tile([128, 256], bf if BF16_T else dt, tag=tag + "hf", name=tag + "hf")
                nc.vector.tensor_mul(hf[:], so[:], tch[:])
                return hf

            def emit_hT(hf, hta, htb, tag):
                idt = identb if BF16_T else identf
                tps = tp.tile([128, 256], bf if BF16_T else dt, tag="tps", name=tag + "tps")
                nc.tensor.transpose(tps[:, 0:128], hf[:, 0:128], idt[:])
                nc.tensor.transpose(tps[:, 128:256], hf[:, 128:256], idt[:])
                nc.vector.tensor_copy(hta[:], tps[:, 0:128])
                nc.vector.tensor_copy(htb[:], tps[:, 128:256])

            def emit_head_reduce(hf2, t):
                if not HEAD_FOLD:
                    return
                # yf = sum(hf2 * whf, axis=free); fold across partition halves
                # happens in emit_head_fold via a tiny constant matmul.
                prod = wk.tile([128, 256], dt, tag="hprod", name=f"t{t}hprod")
                nc.vector.tensor_tensor_reduce(
                    prod[:], hf2[:], whf[:], 1.0, 0.0, ALU.mult, ALU.add, yf[:]
                )

            def emit_head_fold(t):
                hd = tp.tile([BL, 1], dt, tag="hd", name=f"t{t}hd")
                if HEAD_FOLD:
                    # y_all[:, t] = foldm^T @ yf + b_head
                    nc.tensor.matmul(hd[:], foldm[:], yf[:, 0:1], start=True, stop=True)
                else:
                    h2c = h_chunks(h2a, h2b)
                    for k in range(4):
                        nc.tensor.matmul(hd[:], h2c[k], wh4[:, k : k + 1], start=(k == 0), stop=(k == 3))
                nc.vector.tensor_scalar_add(y_all[:, t : t + 1], hd[:], bh64[:, 0:1])

            def emit_decode_feed(t):
                # build xt for step t+1 from y_all[:, t] (consumed next step)
                s = t - (ctx - 1)  # decode step that CONSUMES this pred
                col = hs - 1 - s
                nc.vector.tensor_copy(seq[:, col : col + 1], y_all[:, t : t + 1])
                nc.vector.tensor_copy(xt[:, 0:1], y_all[:, t : t + 1])
                nc.vector.tensor_copy(xt[:, 1:4], covs[:, 3 * s : 3 * s + 3])
                for jj, lag in enumerate(LAGS):
                    src = col + lag
                    nc.vector.tensor_copy(xt[:, 4 + jj : 5 + jj], seq[:, src : src + 1])

            if XPROJ:
                # prefill gates(0)
                emit_xproj(0)

            hf1_prev = None
            hf2_prev = None
            for t in range(nstep):
                enc_step = XPROJ and t < ctx
                # P1: L0 h-recurrence matmuls
                psA0, psB0 = emit_l0_h(t, prefilled=enc_step)
                # P2: h2 transpose + head fold of previous step (stalls hidden under P1)
                if hf2_prev is not None:
                    emit_hT(hf2_prev, h2a, h2b, f"t{t-1}h2")
                    emit_head_fold(t - 1)
                    if t - 1 >= ctx - 1:
                        emit_decode_feed(t - 1)
                # P3: next-step input projection (encoder) / this-step x (decode)
                if enc_step:
                    if t + 1 < ctx:
                        emit_xproj(t + 1)
                elif t < ctx:
                    # XPROJ off: inline encoder input projection
                    xl = enc[:, t * BL : (t + 1) * BL]
                    nc.tensor.matmul(psA0[0:64, :], xl, w0i[:, 0:512], start=False, stop=True, skip_group_check=True)
                    nc.tensor.matmul(psA0[64:128, :], xl, w0i[:, 512:1024], start=False, stop=True, skip_group_check=True)
                    nc.tensor.matmul(psB0[0:64, :], xl, w0i[:, 1024:1536], start=False, stop=True, skip_group_check=True)
                    nc.tensor.matmul(psB0[64:128, :], xl, w0i[:, 1536:2048], start=False, stop=True, skip_group_check=True)
                else:
                    xps = tp.tile([11, BL], bf if BF16_T else dt, tag="tps", name=f"t{t}xps")
                    nc.tensor.transpose(xps[:], xt[:], (identb if BF16_T else identf)[0:BL, 0:BL])
                    nc.vector.tensor_copy(xtT[:], xps[:])
                    emit_l0_x(psA0, psB0)
                # L0 elementwise chain (registers the gate-bank readers
                # BEFORE the L1 bias prefill reuses those banks)
                hf1 = emit_eltwise(psA0, psB0, c1, "l0")
                # P4: L1 bias prefill + h2-half of the L1 contraction
                # (fills the PE gap while the L0 elementwise chain runs)
                psA1, psB1 = emit_l1_bias(t)
                emit_l1_h2part(psA1, psB1)
                # P5: h1 transpose (the step's critical join)
                emit_hT(hf1, h1a, h1b, f"t{t}h1")
                # P6: L1 h1-half
                emit_l1_h1part(psA1, psB1)
                if not BIAS_MM:
                    nc.vector.tensor_add(psA1[:], psA1[:], b1a_f[:])
                    nc.vector.tensor_add(psB1[:], psB1[:], b1b_f[:])
                hf2 = emit_eltwise(psA1, psB1, c2, "l1")
                emit_head_reduce(hf2, t)
                hf1_prev, hf2_prev = hf1, hf2

            if not HEAD_FOLD:
                emit_hT(hf2_prev, h2a, h2b, "final_h2")
            emit_head_fold(nstep - 1)

            nc.vector.tensor_scalar_mul(y_all[:], y_all[:], scale[:, 0:1])
            nc.sync.dma_start(d_y[:], y_all[:])

    nc.finalize()
    return nc


# ---------------------------------------------------------------------------
# Persistent PJRT runner (mirrors bass2jax.run_bass_via_pjrt, but cached so
# repeated calls do not re-trace / re-compile)
# ---------------------------------------------------------------------------


def _make_runner(nc):
    _ensure_path()
    import jax
    from jax.experimental.shard_map import shard_map
    from jax.sharding import Mesh, PartitionSpec

    import concourse.mybir as mybir
    from concourse import bass2jax

    bass2jax.install_neuronx_cc_hook()

    partition_name = nc.partition_id_tensor.name if nc.partition_id_tensor else None
    in_names, out_names, out_avals, zero_shapes = [], [], [], []
    for alloc in nc.m.functions[0].allocations:
        if not isinstance(alloc, mybir.MemoryLocationSet):
            continue
        name = alloc.memorylocations[0].name
        if alloc.kind == "ExternalInput":
            if name != partition_name:
                in_names.append(name)
        elif alloc.kind == "ExternalOutput":
            out_names.append(name)
            shape = tuple(alloc.tensor_shape)
            dtype = mybir.dt.np(alloc.dtype)
            out_avals.append(jax.core.ShapedArray(shape, dtype))
            zero_shapes.append((shape, dtype))
    n_params = len(in_names)
    n_outs = len(out_names)
    all_in = list(in_names) + list(out_names)
    if partition_name is not None:
        all_in.append(partition_name)
    all_in = tuple(all_in)

    def _body(*args):
        operands = list(args)
        if partition_name is not None:
            operands.append(bass2jax.partition_id_tensor())
        outs = bass2jax._bass_exec_p.bind(
            *operands,
            out_avals=tuple(out_avals),
            in_names=all_in,
            out_names=tuple(out_names),
            lowering_input_output_aliases=(),
            sim_require_finite=True,
            sim_require_nnan=True,
            nc=nc,
        )
        return tuple(outs)

    devices = jax.devices()[:NCORES]
    assert len(devices) == NCORES, f"need {NCORES} devices, got {len(jax.devices())}"
    mesh = Mesh(np.asarray(devices), ("core",))
    in_specs = (PartitionSpec("core"),) * (n_params + n_outs)
    out_specs = (PartitionSpec("core"),) * n_outs
    donate = tuple(range(n_params, n_params + n_outs))
    sharded = jax.jit(
        shard_map(_body, mesh=mesh, in_specs=in_specs, out_specs=out_specs, check_rep=False),
        donate_argnums=donate,
        keep_unused=True,
    )

    from jax.sharding import NamedSharding

    sharding = NamedSharding(mesh, PartitionSpec("core"))

    def prepare(in_maps):
        """device_put the concatenated inputs once; reuse across timed calls."""
        concat_in = [
            np.concatenate([np.asarray(in_maps[c][nm]) for c in range(NCORES)], axis=0)
            for nm in in_names
        ]
        return [jax.device_put(a, sharding) for a in concat_in]

    def run_prepared(dev_in):
        concat_zeros = [
            jax.device_put(np.zeros((NCORES * s[0],) + s[1:], d), sharding)
            for (s, d) in zero_shapes
        ]
        out_arrs = sharded(*dev_in, *concat_zeros)
        jax.block_until_ready(out_arrs)
        return out_arrs

    def make_zeros():
        return [
            jax.device_put(np.zeros((NCORES * s[0],) + s[1:], d), sharding)
            for (s, d) in zero_shapes
        ]

    def dispatch(dev_in, zeros):
        return sharded(*dev_in, *zeros)

    def run(in_maps):
        out_arrs = run_prepared(prepare(in_maps))
        outs = []
        for c in range(NCORES):
            outs.append(
                {
                    nm: np.asarray(out_arrs[i]).reshape((NCORES,) + zero_shapes[i][0])[c]
                    for i, nm in enumerate(out_names)
                }
            )
        return outs

    run.prepare = prepare
    run.run_prepared = run_prepared
    run.make_zeros = make_zeros
    run.dispatch = dispatch
    return run


def _get_runner(ctx, hp):
    key = (ctx, hp)
    if key not in _BUILT:
        nc = _build_nc(ctx, hp)
        _BUILT[key] = _make_runner(nc)
    return _BUILT[key]


# ---------------------------------------------------------------------------
# Host-side prep + full model entry
# ---------------------------------------------------------------------------


def _prep_in_maps(X, pad_mask, hp, ctx, W_ih0, W_hh0, b0, W_ih1, W_hh1, b1, W_head, b_head):
    import ml_dtypes

    f32 = _F32
    bf16 = np.dtype(ml_dtypes.bfloat16)
    X = np.asarray(X, f32).copy()
    pad_mask = np.asarray(pad_mask)
    B_, L_, _ = X.shape
    hs = hp - 1
    X[:, L_ - hs :, 0] = 0.0
    past = X[:, : L_ - hs, 0][:, ::-1]  # [B, MAX_LAG+ctx] newest-first
    Xs = X[:, MAX_LAG:]  # [B, ctx+hs, 3]
    m = pad_mask[:, MAX_LAG:][:, :ctx].astype(f32)
    scale = (np.abs(Xs[:, :ctx, 0]) * m).sum(1) / np.maximum(m.sum(1), 1.0)
    scale = np.maximum(scale, 1e-3).astype(f32)  # [B]
    pastn = (past / scale[:, None]).astype(f32)
    logs = np.log(scale)
    tgt = Xs[:, :, 0] / scale[:, None]

    idx = (ctx - 1 - np.arange(ctx))[:, None] + np.asarray(LAGS)[None, :]
    lags = pastn[:, idx]  # [B, ctx, 6]
    enc = np.concatenate(
        [
            tgt[:, :ctx, None],
            Xs[:, :ctx, 1:3],
            np.broadcast_to(logs[:, None, None], (B_, ctx, 1)),
            lags,
            np.ones((B_, ctx, 1), f32),
        ],
        axis=2,
    ).astype(f32)  # [B, ctx, 11]
    covs = np.concatenate(
        [Xs[:, ctx:, 1:3], np.broadcast_to(logs[:, None, None], (B_, hs, 1))], axis=2
    ).astype(f32)  # [B, hs, 3]
    buf0 = pastn[:, : MAX_LAG + 1]

    perm = _gate_perm()
    W_ih0 = np.asarray(W_ih0, f32)[perm]
    W_hh0 = np.asarray(W_hh0, f32)[perm]
    b0p = np.asarray(b0, f32)[perm]
    W_ih1 = np.asarray(W_ih1, f32)[perm]
    W_hh1 = np.asarray(W_hh1, f32)[perm]
    b1p = np.asarray(b1, f32)[perm]
    W_head = np.asarray(W_head, f32)
    b_head = np.asarray(b_head, f32)

    w0i = np.ascontiguousarray(np.concatenate([W_ih0.T, b0p[None, :]], 0))  # [11, G]
    W0hT = W_hh0.T  # [512, G]
    w0h = np.ascontiguousarray(np.concatenate([W0hT[128 * k : 128 * (k + 1)] for k in range(4)], 1))
    W1T = np.concatenate([W_ih1.T, W_hh1.T], 0)  # [1024, G]
    w1 = np.ascontiguousarray(np.concatenate([W1T[128 * k : 128 * (k + 1)] for k in range(8)], 1))

    # L1 bias prefill operands: gates += ind2^T @ b1{A,B}
    ind2 = np.zeros((2, 128), f32)
    ind2[0, 0:64] = 1.0
    ind2[1, 64:128] = 1.0
    b1A = np.stack([b1p[0:512], b1p[512:1024]], 0)  # [2, 512]
    b1B = np.stack([b1p[1024:1536], b1p[1536:2048]], 0)
    b1af = np.empty((128, 512), f32)
    b1af[0:64] = b1p[0:512]
    b1af[64:128] = b1p[512:1024]
    b1bf = np.empty((128, 512), f32)
    b1bf[0:64] = b1p[1024:1536]
    b1bf[64:128] = b1p[1536:2048]

    # head weights in the folded hf2 layout: whf[p, q] = W_head[q + 256*(p>=64)]
    whf = np.empty((128, 256), f32)
    whf[0:64] = W_head[0:256, 0][None, :]
    whf[64:128] = W_head[256:512, 0][None, :]

    bh64 = np.full((BL, 1), float(b_head[0]), f32)
    identb = np.eye(128, dtype=f32)
    foldm = np.zeros((128, BL), f32)
    foldm[np.arange(BL), np.arange(BL)] = 1.0
    foldm[BL + np.arange(BL), np.arange(BL)] = 1.0

    in_maps = []
    for c in range(NCORES):
        sl = slice(c * BL, (c + 1) * BL)
        enc_inT = np.ascontiguousarray(enc[sl].transpose(2, 1, 0).reshape(11, ctx * BL))
        in_maps.append(
            {
                "enc_inT": enc_inT.astype(bf16),
                "w0i": w0i.astype(bf16),
                "w0h": w0h.astype(bf16),
                "w1": w1.astype(bf16),
                "ind2": ind2.astype(bf16),
                "b1A": b1A.astype(bf16),
                "b1B": b1B.astype(bf16),
                "b1af": b1af,
                "b1bf": b1bf,
                "whf": whf,
                "wh4": np.stack([W_head[128 * k : 128 * (k + 1), 0] for k in range(4)], 1).astype(bf16),
                "foldm": foldm,
                "bh64": bh64,
                "covs": np.ascontiguousarray(covs[sl].reshape(BL, max(3 * hs, 1))),
                "buf0": np.ascontiguousarray(buf0[sl]),
                "scale": np.ascontiguousarray(scale[sl, None]),
                "identb": identb.astype(bf16),
                "identf": identb,
            }
        )
    return in_maps, scale


def run_model(X, pad_mask, H, context_length, W_ih0, W_hh0, b0, W_ih1, W_hh1, b1, W_head, b_head):
    hp = int(H)
    ctx = int(context_length)
    in_maps, _ = _prep_in_maps(
        X, pad_mask, hp, ctx, W_ih0, W_hh0, b0, W_ih1, W_hh1, b1, W_head, b_head
    )
    run = _get_runner(ctx, hp)
    outs = run(in_maps)
    y = np.concatenate([outs[c]["y"] for c in range(NCORES)], axis=0)  # [B, nstep]
    return y[:, :, None].astype(_F32)


def kernel(**inputs):
    return run_model(
        inputs["X"],
        inputs["pad_mask"],
        inputs["H"],
        inputs["context_length"],
        inputs["W_ih0"],
        inputs["W_hh0"],
        inputs["b0"],
        inputs["W_ih1"],
        inputs["W_hh1"],
        inputs["b1"],
        inputs["W_head"],
        inputs["b_head"],
    )
